# revision 34
# baseline (speedup 1.0000x reference)
"""Multi-head attention (B=4, S=2048, D=1280, H=10, hd=128) on 8 TRN2 NeuronCores.

Sharding: core c handles batch b = c//2 and heads h0 = 5*(c%2) .. h0+5
(data-parallel over batch x head-parallel tensor parallelism). Host does the
final pairwise all-reduce + bias.

Precision/speed scheme (PE is the bottleneck engine):
  - Projections run as fp8 DoubleRow matmuls (2 contraction k-tiles per pass,
    0.5 cyc/row) with *residual compensation*: operands split hi+lo in e4m3 at
    a fixed power-of-2 scale, cross terms accumulated in fp32 PSUM. V and K
    use 3 terms (x_hi*w_hi + x_lo*w_hi + x_hi*w_lo, ~bf16-grade); Q uses
    2 terms (x compensated, w_q plain fp8) since Q is re-quantized to fp8
    anyway and its noise is dominated by that store.
  - Q,K are stored e4m3 (8x true scale) in the [64, 2, S] split-hd layout
    DoubleRow wants; S^T = K Q^T then runs fp8-DoubleRow at half bf16 cost.
  - P = exp(S*scale) stays bf16 (scale folds all fp8 scaling), O = P V and
    the out-projection stay bf16: their quantization noise would not average
    down (attention output is itself a near-uniform average).
  - V is projected directly transposed (stationary x-tiles, moving w_v),
    removing the baseline's 80 PE transposes and its ACT copy chain.

Schedule: input DMAs all on the SP ring in consumption order; K-proj chunks
interleaved with V-proj token-tiles to track the x stream; Q chunk 0; then
per (chunk, head) attention units with the baseline's jp software pipeline.
The out-projection of chunk ic-1 and the Q-projection of chunk ic+1 run as
PE filler inside the units. Q/K quantization: DVE writes an fp8 staging tile;
two small SBUF->SBUF DMAs on the scalar-HWDGE and Pool-SWDGE rings (kept free
of bulk traffic) move the halves into the [64, 2, ...] layout.
"""

import numpy as np

B, S, D = 4, 2048, 1280
HEADS = 10
HD = 128
NH = 5              # heads per core
P = 128
SCALE = float(D) ** -0.5
KT_D = D // P       # 10 k-tiles over D
KTP = KT_D // 2     # 5 DoubleRow k-tile pairs
NJT = S // P        # 16 j tiles
NIC = S // 512      # 4 i-chunks of 512
CX = 4.0            # x fp8 scale
CW = 64.0           # w fp8 scale
CQK = 1.0 / 32.0    # Q/K store rescale: psum 256x -> stored 8x true
EXP_SCALE = SCALE / 64.0   # dots psum carries (8*8)=64x true scale
CV = 1.0 / 256.0    # V store rescale: psum 256x -> true

_PROGRAM_CACHE = {}


def _build_program(repeat=1):
    if repeat in _PROGRAM_CACHE:
        return _PROGRAM_CACHE[repeat]

    import concourse.mybir as mybir
    from concourse import bacc
    import concourse.tile as tile

    F32 = mybir.dt.float32
    F32R = mybir.dt.float32r
    BF16 = mybir.dt.bfloat16
    F8 = mybir.dt.float8e4
    EXP = mybir.ActivationFunctionType.Exp
    DR = mybir.MatmulPerfMode.DoubleRow

    nc = bacc.Bacc()
    xh_d = nc.declare_dram_parameter("xh", [D, S], F8, isOutput=False)
    xl_d = nc.declare_dram_parameter("xl", [D, S], F8, isOutput=False)
    wqkh_d = nc.declare_dram_parameter("wqkh", [D, 2 * NH * HD], F8, isOutput=False)
    wkl_d = nc.declare_dram_parameter("wkl", [D, NH * HD], F8, isOutput=False)
    wvh_d = nc.declare_dram_parameter("wvh", [D, NH * HD], F8, isOutput=False)
    wvl_d = nc.declare_dram_parameter("wvl", [D, NH * HD], F8, isOutput=False)
    wout_d = nc.declare_dram_parameter("wout", [NH * HD, D], BF16, isOutput=False)
    onesr_d = nc.declare_dram_parameter("onesr_in", [1, P], F32, isOutput=False)
    out_d = nc.declare_dram_parameter("outT", [D, S], F32, isOutput=True)

    xh_t = xh_d[:].rearrange("(kt p) s -> p kt s", p=P)        # [128, 10, 2048]
    xl_t = xl_d[:].rearrange("(kt p) s -> p kt s", p=P)
    wqkh_t = wqkh_d[:].rearrange("(kt p) m -> p kt m", p=P)    # [128, 10, 1280]
    wkl_t = wkl_d[:].rearrange("(kt p) m -> p kt m", p=P)      # [128, 10, 640]
    wvh_t = wvh_d[:].rearrange("(kt p) m -> p kt m", p=P)
    wvl_t = wvl_d[:].rearrange("(kt p) m -> p kt m", p=P)
    wout_t = wout_d[:].rearrange("(kt p) m -> p kt m", p=P)    # [128, 5, 1280]

    with tile.TileContext(nc) as tc:
        with (
            tc.tile_pool(name="persist", bufs=1) as persist,
            tc.tile_pool(name="oio", bufs=2) as oio,
            tc.tile_pool(name="work", bufs=4) as work,
            tc.tile_pool(name="ptp", bufs=6) as ptp,
            tc.tile_pool(name="work2", bufs=2) as work2,
            tc.tile_pool(name="stgp", bufs=4) as stgp,
            tc.tile_pool(name="ps_mm", bufs=2, space="PSUM") as ps_mm,
            tc.tile_pool(name="ps_acc", bufs=2, space="PSUM") as ps_acc,
            tc.tile_pool(name="ps_sm", bufs=2, space="PSUM") as ps_sm,
        ):
            XH = persist.tile([P, KT_D, S], F8, name="XH")
            XL = persist.tile([P, KT_D, S], F8, name="XL")
            WQKH = persist.tile([P, KT_D, 2 * NH * HD], F8, name="WQKH")
            WKL = persist.tile([P, KT_D, NH * HD], F8, name="WKL")
            WVH = persist.tile([P, KT_D, NH * HD], F8, name="WVH")
            WVL = persist.tile([P, KT_D, NH * HD], F8, name="WVL")
            WO = persist.tile([P, NH, D], BF16, name="WO")
            QS = persist.tile([64, 2, NH, S], F8, name="QS")
            KS = persist.tile([64, 2, NH, S], F8, name="KS")
            V = persist.tile([P, NJT, NH * HD], BF16, name="V")
            ones = persist.tile([P, 1], BF16, name="ones")
            onesr = persist.tile([1, P], F32R, name="onesr")

            nc.gpsimd.memset(ones[:], 1.0)

            def load_inputs():
                # Everything on the SP ring, in consumption order, so the
                # (globally serialized) DMA engines feed the lead-in without
                # the small latency-critical shift DMAs queueing behind bulk.
                def w_m(m):
                    msl = slice(m * P, (m + 1) * P)
                    nc.sync.dma_start(WQKH[:, :, msl], wqkh_t[:, :, msl])

                def kl_m(m):
                    msl = slice(m * P, (m + 1) * P)
                    nc.sync.dma_start(WKL[:, :, msl], wkl_t[:, :, msl])

                def x_ic(ic):
                    isl = slice(ic * 512, (ic + 1) * 512)
                    nc.sync.dma_start(XH[:, :, isl], xh_t[:, :, isl])
                    nc.sync.dma_start(XL[:, :, isl], xl_t[:, :, isl])

                nc.sync.dma_start(onesr[:], onesr_d[:].bitcast(F32R))
                w_m(NH)
                kl_m(0)
                x_ic(0)
                for m in range(1, NH):
                    w_m(NH + m)
                    kl_m(m)
                x_ic(1)
                nc.sync.dma_start(WVH[:], wvh_t)
                nc.sync.dma_start(WVL[:], wvl_t)
                x_ic(2)
                for m in range(NH):
                    w_m(m)       # Q columns
                x_ic(3)
                nc.sync.dma_start(WO[:], wout_t)

            def proj_qk(m, ic, dst, h):
                """One [128,512] Q or K projection tile -> fp8 into dst[64,2,...].

                K (dst is KS) adds the w_lo cross term; Q skips it."""
                isl = slice(ic * 512, (ic + 1) * 512)
                msl = slice(m * P, (m + 1) * P)
                kmsl = slice((m - NH) * P, (m - NH + 1) * P)
                terms = ((WQKH, msl, XH), (WQKH, msl, XL))
                if dst is KS:
                    terms += ((WKL, kmsl, XH),)
                q_ps = ps_sm.tile([P, 512], F32, name="sm")
                nterm = len(terms) * KTP
                step = 0
                for ktp in range(KTP):
                    k2 = slice(2 * ktp, 2 * ktp + 2)
                    for Wt, wsl, Xt in terms:
                        nc.tensor.matmul(
                            q_ps[:], Wt[:, k2, wsl], Xt[:, k2, isl],
                            start=(step == 0), stop=(step == nterm - 1),
                            perf_mode=DR,
                        )
                        step += 1
                stg = stgp.tile([P, 512], F8, name="stg")
                nc.vector.tensor_scalar_mul(stg[:], q_ps[:], CQK)
                # layout-shift DMAs ride rings with no bulk traffic
                nc.scalar.dma_start(dst[:, 0, h, isl], stg[0:64, :])
                nc.gpsimd.dma_start(dst[:, 1, h, isl], stg[64:128, :])

            def proj_v(tt):
                """V rows for token-tile tt, direct-transposed: [128 tok, 640]."""
                tsl = slice(tt * P, (tt + 1) * P)
                v_ps = ps_mm.tile([P, 1024], F32, name="mm")
                step = 0
                for ktp in range(KTP):
                    k2 = slice(2 * ktp, 2 * ktp + 2)
                    for Xt, Wt in ((XH, WVH), (XH, WVL), (XL, WVH)):
                        st = (step == 0)
                        sp = (step == 3 * KTP - 1)
                        nc.tensor.matmul(v_ps[:, 0:512], Xt[:, k2, tsl],
                                         Wt[:, k2, 0:512], start=st, stop=sp,
                                         perf_mode=DR)
                        nc.tensor.matmul(v_ps[:, 512:640], Xt[:, k2, tsl],
                                         Wt[:, k2, 512:640], start=st, stop=sp,
                                         perf_mode=DR)
                        step += 1
                nc.vector.tensor_scalar_mul(V[:, tt], v_ps[:, 0:640], CV)

            for rep in range(repeat):
                if rep == 0:
                    load_inputs()

                # ---- lead-in: K chunks + V token-tiles track the x stream ---
                for m in range(NH):
                    proj_qk(NH + m, 0, KS, m)
                for m in range(NH):
                    proj_qk(NH + m, 1, KS, m)
                for tt in range(4):
                    proj_v(tt)
                for m in range(NH):
                    proj_qk(NH + m, 2, KS, m)
                for tt in range(4, 8):
                    proj_v(tt)
                for m in range(NH):
                    proj_qk(NH + m, 3, KS, m)
                for tt in range(8, 12):
                    proj_v(tt)
                for m in range(NH):
                    proj_qk(m, 0, QS, m)
                for tt in range(12, NJT):
                    proj_v(tt)

                # ---- attention + out projection -----------------------------
                def norm_tail(st):
                    fold, o_ps, OT, h = st
                    fh = work2.tile([P, 512], BF16, name="fh", tag="fh")
                    nc.vector.tensor_add(fh[:], fold[:, :512], fold[:, 512:])
                    sum_ps = ps_sm.tile([P, 512], F32, name="sm")[0:1, :]
                    nc.tensor.matmul(sum_ps, ones[:], fh[:],
                                     start=True, stop=True)
                    s_row = work2.tile([1, 512], F32R, name="s_row", tag="s_row")
                    nc.vector.tensor_copy(s_row[:], sum_ps)
                    bc_ps = ps_sm.tile([P, 512], F32, name="sm")
                    nc.tensor.matmul(bc_ps[:], onesr[:], s_row[:],
                                     start=True, stop=True)
                    rec = work2.tile([P, 512], F32, name="rec", tag="rec")
                    nc.vector.reciprocal(rec[:], bc_ps[:])
                    nc.vector.tensor_mul(OT[:, h, :], o_ps[:], rec[:])

                def out_proj(ic, OT, ms, alt=False):
                    isl = slice(ic * 512, (ic + 1) * 512)
                    for i, m in enumerate(ms):
                        p_ps = ps_sm.tile([P, 512], F32, name="sm")
                        for kt in range(NH):
                            nc.tensor.matmul(
                                p_ps[:], WO[:, kt, m * P:(m + 1) * P], OT[:, kt, :],
                                start=(kt == 0), stop=(kt == NH - 1),
                            )
                        outc = work.tile([P, 512], F32, name="outc")
                        nc.vector.tensor_copy(outc[:], p_ps[:])
                        nc.sync.dma_start(out_d[m * P:(m + 1) * P, isl], outc[:])

                pending_tail = None
                pending_proj = None
                for ic in range(NIC):
                    isl = slice(ic * 512, (ic + 1) * 512)
                    OT = oio.tile([P, NH, 512], BF16, name="OT")
                    for h in range(NH):
                        fold = work2.tile([P, 1024], BF16, name="fold", tag="fold")
                        o_ps = ps_acc.tile([P, 512], F32, name="acc")
                        pt2s = [None] * (NJT // 2)
                        # software-pipelined: the paired S-DR-matmuls + one
                        # wide exp run a pair ahead of the O-matmuls so PE
                        # never waits on ACT.
                        for jp in range(NJT // 2 + 2):
                            if jp < NJT // 2:
                                s_ps = ps_mm.tile([P, 1024], F32, name="mm")
                                for half in range(2):
                                    jt = 2 * jp + half
                                    jsl = slice(jt * P, (jt + 1) * P)
                                    nc.tensor.matmul(
                                        s_ps[:, half * 512:(half + 1) * 512],
                                        KS[:, :, h, jsl], QS[:, :, h, isl],
                                        start=True, stop=True, perf_mode=DR,
                                    )
                                pt2 = ptp.tile([P, 1024], BF16, name="pt")
                                nc.scalar.activation(pt2[:], s_ps[:], EXP,
                                                     scale=EXP_SCALE)
                                pt2s[jp] = pt2
                                if jp == 1:
                                    nc.vector.tensor_add(
                                        fold[:], pt2s[0][:], pt2s[1][:])
                                elif jp > 1:
                                    nc.vector.tensor_add(fold[:], fold[:], pt2[:])
                            if jp > 1:
                                prev = pt2s[jp - 2]
                                for half in range(2):
                                    jt = 2 * (jp - 2) + half
                                    nc.tensor.matmul(
                                        o_ps[:], V[:, jt, h * P:(h + 1) * P],
                                        prev[:, half * 512:(half + 1) * 512],
                                        start=(jt == 0), stop=(jt == NJT - 1),
                                    )
                            if jp == 1 and pending_tail is not None:
                                norm_tail(pending_tail)
                                pending_tail = None
                            if jp in (3, 6) and pending_proj is not None:
                                pic, pOT = pending_proj
                                m0 = 2 * h + (0 if jp == 3 else 1)
                                out_proj(pic, pOT, [m0], alt=(jp == 6))
                                if h == NH - 1 and jp == 6:
                                    pending_proj = None
                            if jp == 4 and ic < NIC - 1:
                                # Q projection of the next chunk as PE filler
                                proj_qk(h, ic + 1, QS, h)
                        pending_tail = (fold, o_ps, OT, h)
                    pending_proj = (ic, OT)
                norm_tail(pending_tail)
                out_proj(*pending_proj, range(D // P))

    nc.finalize()
    _PROGRAM_CACHE[repeat] = nc
    return nc


def _enc_hi_lo(a, scale):
    """Split scale*a into e4m3 hi + lo (same scale; lo holds the residual)."""
    import ml_dtypes
    f8 = ml_dtypes.float8_e4m3
    sa = np.asarray(a, np.float32) * scale
    hi = sa.astype(f8)
    lo = (sa - hi.astype(np.float32)).astype(f8)
    return hi, lo


def _shard_inputs(x, w_qkv, w_out):
    """Build the 8 per-core input maps (fp8 hi/lo operands, host-encoded)."""
    import ml_dtypes
    bf16 = ml_dtypes.bfloat16
    onesr = np.ones((1, P), np.float32)
    in_maps = []
    for c in range(8):
        b = c // 2
        h0 = NH * (c % 2)
        xT = np.ascontiguousarray(np.asarray(x[b], np.float32).T)      # [D, S]
        xh, xl = _enc_hi_lo(xT, CX)
        qk = np.concatenate([
            w_qkv[:, qi * D + h0 * HD: qi * D + (h0 + NH) * HD] for qi in range(2)
        ], axis=1)                                                     # [D, 1280]
        wqkh, wqkl = _enc_hi_lo(qk, CW)
        wv = w_qkv[:, 2 * D + h0 * HD: 2 * D + (h0 + NH) * HD]         # [D, 640]
        wvh, wvl = _enc_hi_lo(wv, CW)
        in_maps.append(dict(
            xh=xh, xl=xl, wqkh=wqkh,
            wkl=np.ascontiguousarray(wqkl[:, NH * HD:]),   # K columns' lo part
            wvh=wvh, wvl=wvl,
            wout=np.ascontiguousarray(
                np.asarray(w_out[h0 * HD:(h0 + NH) * HD, :], np.float32)
            ).astype(bf16),
            onesr_in=onesr,
        ))
    return in_maps


def run_sharded(x, w_qkv, w_out, b_out, repeat=1, trace=False):
    """Run the SPMD program; returns (out [B,S,D], BassKernelResults)."""
    from concourse.bass_utils import run_bass_kernel_spmd

    nc = _build_program(repeat)
    in_maps = _shard_inputs(x, w_qkv, w_out)
    res = run_bass_kernel_spmd(nc, in_maps, list(range(8)), trace=trace)
    out = np.empty((B, S, D), np.float32)
    for b in range(B):
        out[b] = (res.results[2 * b]["outT"].T
                  + res.results[2 * b + 1]["outT"].T
                  + b_out[None, :])
    return out, res


def kernel(x, w_qkv, w_out, b_out):
    x = np.asarray(x, np.float32)
    w_qkv = np.asarray(w_qkv, np.float32)
    w_out = np.asarray(w_out, np.float32)
    b_out = np.asarray(b_out, np.float32)
    out, _ = run_sharded(x, w_qkv, w_out, b_out)
    return out
